# revision 1
# baseline (speedup 1.0000x reference)
"""MatchLSTM attention kernel for 8 Trainium2 NeuronCores.

Reference computation (B=64, T=2048, D=512):
    G   = tanh(input_p@Wp.T + bp + input_q@Wq.T + bq + h_tm1@Wr.T + br)
    a   = softmax(G@w + match_b)            over T
    z   = sum_t a[:,t] * input_q[:,:,t]
    out = concat([input_p, z], -1)

Sharding: data-parallel over batch, 8 batches per core, weights replicated.

Per-core device pipeline (all matmul operands bf16, fp32 accumulation):
  - c^T[o,b] = (Wp.T;Wr.T;bias) matmuls against (input_p^T;h^T;ones)  [once]
  - X^T tiles [q,tok] via DMA-transpose; X natural tiles [tok,q] via DMA
  - G^T[o,tok] = Wq.T-chunk @ X^T-chunk (PE, fp32 PSUM)
  - tanh via ScalarE with per-partition bias c^T  -> bf16 SBUF
  - scores s[1,tok] = w-chunk.T @ tanhG (PE accum over o-chunks)
  - s transposed to columns via K=1 fp16 matmuls; exp(s+match_b) on ScalarE
    -> bf16, with sumexp accumulated for free via activation accum_out
  - z[1,512] = sum_j esc_j.T @ Xnat_j (PE, fp32 PSUM accumulation)
  - z scaled by 1/sumexp (VectorE), DMA out.  Softmax max-subtraction is
    skipped: |s| <= sum|w| + 1 < 25, exp stays well inside fp32 range.
"""

import sys

if "/opt/trn_rl_repo" not in sys.path:
    sys.path.insert(0, "/opt/trn_rl_repo")

import numpy as np
import ml_dtypes

N_CORES = 8
B, T, D = 64, 2048, 512
PB = B // N_CORES          # batches per core
KC = D // 128              # 4 contraction chunks of 128
NTT = T // 512             # 4 token tiles of 512
NJ = T // 128              # 16 token chunks of 128
CROWS = 2 * D + 128        # cw/cx rows: Wp.T, Wr.T, bias row + zero pad

BF16 = ml_dtypes.bfloat16

_CACHE: dict = {}


def _build_program():
    import concourse.bacc as bacc
    import concourse.tile as tile
    import concourse.mybir as mybir
    from concourse.bass import MemorySpace

    dt = mybir.dt
    F32 = dt.float32
    BF = dt.bfloat16
    AF = mybir.ActivationFunctionType

    nc = bacc.Bacc(
        "TRN2", target_bir_lowering=False, debug=False, num_devices=N_CORES
    )

    xq_d = nc.dram_tensor("xq", [PB, T, D], BF, kind="ExternalInput")
    wq_d = nc.dram_tensor("wqt", [D, D], BF, kind="ExternalInput")      # Wq.T [q,o]
    cw_d = nc.dram_tensor("cw", [CROWS, D], BF, kind="ExternalInput")   # [Wp.T;Wr.T;bias;0]
    cx_d = nc.dram_tensor("cx", [CROWS, PB], BF, kind="ExternalInput")  # [ip.T;h.T;1;0]
    wcol_d = nc.dram_tensor("wcol", [D, 1], BF, kind="ExternalInput")
    mb_d = nc.dram_tensor("mb", [128, 1], F32, kind="ExternalInput")    # match_b bcast
    z_d = nc.dram_tensor("z", [1, PB * D], F32, kind="ExternalOutput")

    NKC = CROWS // 128  # 9 contraction chunks for the c matmuls

    F16 = dt.float16

    with tile.TileContext(nc) as tc:
        with (
            tc.tile_pool(name="consts", bufs=1) as consts,
            tc.tile_pool(name="xT_p", bufs=3) as xT_pool,
            tc.tile_pool(name="xnat_p", bufs=3) as xnat_pool,
            tc.tile_pool(name="tanh_p", bufs=8) as tanh_pool,
            tc.tile_pool(name="srow_p", bufs=3) as srow_pool,
            tc.tile_pool(name="esc_p", bufs=3) as esc_pool,
            tc.tile_pool(name="small_p", bufs=2) as small_pool,
            tc.tile_pool(name="zout_p", bufs=1) as zout_pool,
            tc.tile_pool(name="pG", bufs=2, space=MemorySpace.PSUM) as pG,
            tc.tile_pool(name="pS", bufs=2, space=MemorySpace.PSUM) as pS,
            tc.tile_pool(name="pZ", bufs=1, space=MemorySpace.PSUM) as pZ,
            tc.tile_pool(name="pM", bufs=1, space=MemorySpace.PSUM) as pM,
        ):
            # ---- constants (DMA order = criticality order) -----------------
            cw_s = consts.tile([128, NKC, D], BF, tag="cw", name="cw_s")
            nc.sync.dma_start(out=cw_s, in_=cw_d.rearrange("(c p) o -> p c o", p=128))
            cx_s = consts.tile([128, NKC, PB], BF, tag="cx", name="cx_s")
            nc.sync.dma_start(out=cx_s, in_=cx_d.rearrange("(c p) b -> p c b", p=128))
            wq_s = consts.tile([128, KC, D], BF, tag="wq", name="wq_s")
            nc.sync.dma_start(out=wq_s, in_=wq_d.rearrange("(c p) o -> p c o", p=128))
            wcol_s = consts.tile([128, KC, 1], BF, tag="wcol", name="wcol_s")
            nc.sync.dma_start(out=wcol_s, in_=wcol_d.rearrange("(c p) o -> p c o", p=128))
            mb_s = consts.tile([128, 1], F32, tag="mb", name="mb_s")
            nc.sync.dma_start(out=mb_s, in_=mb_d[:, :])
            ones128 = consts.tile([128, 1], F32, tag="ones128", name="ones128")
            nc.vector.memset(ones128, 1.0)
            ones_f16 = consts.tile([1, 1], F16, tag="ones_f16", name="ones_f16")
            nc.vector.memset(ones_f16, 1.0)
            # warm the ACT table set (tanh/exp share one set) off the critical path
            dummy_s = consts.tile([1, 1], F32, tag="dummy", name="dummy_s")
            nc.scalar.activation(
                out=dummy_s, in_=ones_f16, func=AF.Tanh, bias=0.0, scale=1.0
            )

            # ---- c^T[o, b] for all batches (once) --------------------------
            c_ps = pM.tile([128, KC, PB], F32, tag="misc", name="c_ps")
            for oc in range(KC):
                for k in range(NKC):
                    nc.tensor.matmul(
                        c_ps[:, oc, :],
                        cw_s[:, k, oc * 128 : (oc + 1) * 128],
                        cx_s[:, k, :],
                        start=(k == 0),
                        stop=(k == NKC - 1),
                    )
            cT_s = consts.tile([128, KC, PB], F32, tag="cT", name="cT_s")
            nc.vector.tensor_copy(out=cT_s, in_=c_ps)

            zout_s = zout_pool.tile([1, PB, D], F32, tag="zout", name="zout_s")

            # ---- per-batch pipeline ---------------------------------------
            for b in range(PB):
                xT = xT_pool.tile([128, KC, T], BF, tag="xT", name="xT")
                # batch 0 is latency-critical: land the first half-T of each
                # q-chunk sooner by splitting the transposes.
                nh = 2 if b == 0 else 1
                for h in range(nh):
                    for qc in range(KC):
                        nc.sync.dma_start(
                            out=xT[:, qc, h * (T // nh) : (h + 1) * (T // nh)],
                            in_=xq_d[
                                b,
                                h * (T // nh) : (h + 1) * (T // nh),
                                qc * 128 : (qc + 1) * 128,
                            ],
                            transpose=True,
                        )
                xnat = xnat_pool.tile([128, NJ, D], BF, tag="xnat", name="xnat")
                nc.sync.dma_start(
                    out=xnat, in_=xq_d[b].rearrange("(i p) q -> p i q", p=128)
                )

                s_cat = srow_pool.tile([1, T], F16, tag="scat", name="s_cat")
                esc = esc_pool.tile([128, NJ], BF, tag="esc", name="esc")
                pesum = small_pool.tile([128, 2], F32, tag="pesum", name="pesum")
                z_ps = pZ.tile([1, D], F32, tag="z", name="z_ps")
                # token tiles processed in pairs sharing one [128,1024] PSUM
                # G tile (2 banks): same Wq chunk stays loaded across the pair
                # and tanh runs once per 1024 tokens.
                for tp in range(NTT // 2):
                    sT_ps = pM.tile([128, NJ // 2], F32, tag="misc", name="sT_ps")
                    sc_pair = [
                        pS.tile([1, 512], F32, tag="s", name="sc_ps")
                        for _ in range(2)
                    ]
                    for oc in range(KC):
                        g_ps = pG.tile([128, 1024], F32, tag="g", name="g_ps")
                        for qc in range(KC):
                            for i in range(2):
                                tt = tp * 2 + i
                                nc.tensor.matmul(
                                    g_ps[:, i * 512 : (i + 1) * 512],
                                    wq_s[:, qc, oc * 128 : (oc + 1) * 128],
                                    xT[:, qc, tt * 512 : (tt + 1) * 512],
                                    start=(qc == 0),
                                    stop=(qc == KC - 1),
                                )
                        th = tanh_pool.tile([128, 1024], BF, tag="th", name="th")
                        nc.scalar.activation(
                            out=th,
                            in_=g_ps,
                            func=AF.Tanh,
                            bias=cT_s[:, oc, b : b + 1],
                            scale=1.0,
                        )
                        for i in range(2):
                            nc.tensor.matmul(
                                sc_pair[i],
                                wcol_s[:, oc, :],
                                th[:, i * 512 : (i + 1) * 512],
                                start=(oc == 0),
                                stop=(oc == KC - 1),
                            )
                    for i in range(2):
                        tt = tp * 2 + i
                        nc.vector.tensor_copy(
                            out=s_cat[:, tt * 512 : (tt + 1) * 512], in_=sc_pair[i]
                        )
                        # transpose scores into columns (K=1 fp16 matmuls)
                        for jj in range(4):
                            j = tt * 4 + jj
                            nc.tensor.matmul(
                                sT_ps[:, j - tp * 8 : j - tp * 8 + 1],
                                s_cat[:, j * 128 : (j + 1) * 128],
                                ones_f16,
                                start=True,
                                stop=True,
                            )
                    # exp + its half of the z accumulation start mid-batch
                    nc.scalar.activation(
                        out=esc[:, tp * 8 : (tp + 1) * 8],
                        in_=sT_ps,
                        func=AF.Exp,
                        bias=mb_s,
                        scale=1.0,
                        accum_out=pesum[:, tp : tp + 1],
                    )
                    for j in range(tp * 8, (tp + 1) * 8):
                        nc.tensor.matmul(
                            z_ps,
                            esc[:, j : j + 1],
                            xnat[:, j, :],
                            start=(j == 0),
                            stop=(j == NJ - 1),
                        )

                se_ps = pM.tile([1, 2], F32, tag="misc", name="se_ps")
                nc.tensor.matmul(se_ps, ones128, pesum, start=True, stop=True)
                se_sb = small_pool.tile([1, 2], F32, tag="sesb", name="se_sb")
                nc.vector.tensor_copy(out=se_sb, in_=se_ps)
                se_tot = small_pool.tile([1, 1], F32, tag="setot", name="se_tot")
                nc.vector.tensor_add(se_tot, se_sb[:, 0:1], se_sb[:, 1:2])
                rse_s = small_pool.tile([1, 1], F32, tag="rse", name="rse_s")
                nc.vector.reciprocal(out=rse_s, in_=se_tot)

                nc.vector.tensor_scalar_mul(
                    out=zout_s[:, b, :], in0=z_ps, scalar1=rse_s
                )

            nc.sync.dma_start(out=z_d[:, :], in_=zout_s)

    nc.compile()
    return nc


def _get_program():
    if "nc" not in _CACHE:
        _CACHE["nc"] = _build_program()
    return _CACHE["nc"]


def kernel(**inputs) -> np.ndarray:
    from concourse import bass_utils

    inp = {k: np.asarray(v) for k, v in inputs.items()}
    input_p = inp["input_p"].astype(np.float32)
    input_q = inp["input_q"].astype(np.float32)
    h_tm1 = inp["h_tm1"].astype(np.float32)
    Wp, Wq, Wr = inp["Wp"], inp["Wq"], inp["Wr"]
    bp, bq, br = inp["bp"], inp["bq"], inp["br"]
    w = inp["w"]
    mb = float(np.asarray(inp["match_b"]).reshape(-1)[0])

    # shared (weight) tensors
    wqt = np.ascontiguousarray(Wq.T).astype(BF16)
    cw = np.zeros((CROWS, D), dtype=BF16)
    cw[:D] = Wp.T.astype(BF16)
    cw[D : 2 * D] = Wr.T.astype(BF16)
    cw[2 * D] = (bp.astype(np.float32) + bq + br).astype(BF16)
    wcol = np.ascontiguousarray(w.reshape(D, 1)).astype(BF16)
    mb_arr = np.full((128, 1), mb, dtype=np.float32)

    nc = _get_program()

    in_maps = []
    for c in range(N_CORES):
        s = slice(c * PB, (c + 1) * PB)
        cx = np.zeros((CROWS, PB), dtype=BF16)
        cx[:D] = input_p[s].T.astype(BF16)
        cx[D : 2 * D] = h_tm1[s].T.astype(BF16)
        cx[2 * D] = 1.0
        in_maps.append(
            {
                "xq": np.ascontiguousarray(input_q[s]).astype(BF16),
                "wqt": wqt,
                "cw": cw,
                "cx": cx,
                "wcol": wcol,
                "mb": mb_arr,
            }
        )

    res = bass_utils.run_bass_kernel_spmd(
        nc, in_maps, core_ids=list(range(N_CORES))
    )
    z = np.concatenate(
        [
            np.asarray(res.results[c]["z"], dtype=np.float32).reshape(PB, D)
            for c in range(N_CORES)
        ],
        axis=0,
    )
    return np.concatenate([input_p, z], axis=1)

